# revision 27
# baseline (speedup 1.0000x reference)
"""Fused Conv3x3 + BatchNorm(train) + ReLU on 8 TRN2 NeuronCores.

Data-parallel over batch: each core processes 8 of the 64 images.
Conv is computed as matmuls over PSUM tiles of [128 out_ch, 512 pixels]:
the 9 filter taps are covered per tile by 3 K=128 matmuls (kh=0,1 paired
on the partition axis) plus 3 K=64 matmuls (kh=2) that are row-tiled so
two of them run concurrently in the PE array (rows 0-63 / rows 64-127)
-> ~5 effective 512-px streaming slots per tile instead of 6.

y is kept resident in SBUF as bf16. Per group the PSUM drain is split:
ScalarE copies PSUM->ybuf(bf16) with accum_out giving per-channel sums;
VectorE squares the bf16 y (2x DVE rate) and reduces to sum-of-squares.

Cross-core BN stats use an AllGather (floor ~5us vs ~27us measured for
AllReduce) followed by a local 3-step tree add. Scale/shift use a single
Rsqrt activation. Apply = relu(y*scale+shift) on ScalarE in chunks,
DMA-out overlapped (the tail is HBM-write-bound).
"""

import os

import numpy as np

import concourse.bacc as bacc
import concourse.tile as tile
from concourse import mybir
from concourse.bass_utils import run_bass_kernel_spmd

N_CORES = 8
IMG_PER_CORE = 8          # 64 images / 8 cores
C_IN = 64
C_OUT = 128
H = W = 64
HP, WP = H + 2, W + 2     # zero-padded image
PIX = H * W               # 4096
TILE_PX = 512             # one PSUM bank of fp32
ROWS_PER_TILE = TILE_PX // W       # 8
TILES_PER_IMG = PIX // TILE_PX     # 8
N_TILES = IMG_PER_CORE * TILES_PER_IMG  # 64
BN_EPS = 1e-5
COUNT = 64 * H * W        # batch-stat count over (N, H, W)

F32 = mybir.dt.float32
BF16 = mybir.dt.bfloat16

# Set by test harness to capture a profile; LAST_EXEC_NS holds the result.
KERNEL_TRACE = False
LAST_EXEC_NS = None
LAST_TRACE_PATH = None
LAST_PROFILE_JSON = None

_cached_nc = None

# bisect toggles (harness leaves these at defaults)
USE_TILED = os.environ.get("K_TILED", "1") == "1"   # row-tiled kh=2 taps
USE_AG = os.environ.get("K_AG", "1") == "1"         # AllGather vs AllReduce
USE_BF16Y = os.environ.get("K_BF16Y", "1") == "1"   # bf16 ybuf
SPLIT_AG = os.environ.get("K_SPLIT_AG", "0") == "1"  # early partial AG


def _build():
    nc = bacc.Bacc("TRN2", target_bir_lowering=False, debug=False,
                   num_devices=N_CORES)

    x_in = nc.dram_tensor("x", [IMG_PER_CORE, 128, HP * WP], BF16,
                          kind="ExternalInput")
    wt_in = nc.dram_tensor("wt", [128, 6, 128], BF16, kind="ExternalInput")
    gb_in = nc.dram_tensor("gb", [128, 2], F32, kind="ExternalInput")
    out_d = nc.dram_tensor("out", [IMG_PER_CORE, C_OUT, PIX], F32,
                           kind="ExternalOutput")
    cc_in = nc.dram_tensor("cc_in", [128, 2], F32)
    cc_out = nc.dram_tensor("cc_out", [N_CORES, 128, 2], F32,
                            addr_space="Shared")
    cc_ar = nc.dram_tensor("cc_ar", [128, 2], F32, addr_space="Shared")
    cc_in_a = nc.dram_tensor("cc_in_a", [128, 2], F32)
    cc_out_a = nc.dram_tensor("cc_out_a", [N_CORES, 128, 2], F32,
                              addr_space="Shared")

    with tile.TileContext(nc) as tc:
        with (
            tc.tile_pool(name="consts", bufs=1) as consts,
            tc.tile_pool(name="xx", bufs=2) as xx_pool,
            tc.tile_pool(name="ybuf", bufs=1) as ybuf_pool,
            tc.tile_pool(name="scratch", bufs=2) as scratch_pool,
            tc.tile_pool(name="stats", bufs=1) as stats_pool,
            tc.tile_pool(name="outp", bufs=3) as out_pool,
            tc.tile_pool(name="psum", bufs=2, space="PSUM") as psum_pool,
        ):
            wt = consts.tile([128, 6, 128], BF16)
            nc.sync.dma_start(out=wt[:], in_=wt_in[:])
            gb = consts.tile([128, 2], F32)
            nc.sync.dma_start(out=gb[:], in_=gb_in[:])
            eps_t = consts.tile([128, 1], F32)
            nc.vector.memset(eps_t[:], BN_EPS)

            # y stays resident in SBUF (bf16) between conv and BN apply.
            YDT = BF16 if USE_BF16Y else F32
            ybuf = ybuf_pool.tile([128, N_TILES, TILE_PX], YDT)
            sums = stats_pool.tile([128, N_TILES // 4], F32)
            sumsqs = stats_pool.tile([128, N_TILES // 4], F32)

            for img in range(IMG_PER_CORE):
                # xx: padded image, channels on partitions 0-63; partitions
                # 64-127 hold the same image shifted down one padded row so
                # (kh, kh+1) taps pair into one K=128 contraction.
                xx = xx_pool.tile([128, HP, WP], BF16)
                # split at the half-image boundary (row 35): the hf=0
                # matmuls only read rows 0-34, so they can start as soon
                # as the first chunk lands. Two HWDGE queues in parallel.
                cut = 35 * WP
                xv = xx[:, :, :].rearrange("p a b -> p (a b)")
                nc.sync.dma_start(out=xv[:, 0:cut], in_=x_in[img, :, 0:cut])
                nc.sync.dma_start(out=xv[:, cut:HP * WP],
                                  in_=x_in[img, :, cut:HP * WP])

                # Singles (kh=2) first so K never shrinks within a PSUM
                # bank. All kh=2 taps are readable from either image copy,
                # so output tiles 0-1 take theirs from the lower copy (PE
                # rows 0-63) and tiles 2-3 from the row-shifted upper copy
                # (rows 64-127): disjoint row groups AND disjoint PSUM
                # banks, so the PE streams both concurrently (race-free).
                # Issue order interleaves the two row groups.
                for hf in range(2):
                    gh = img * 2 + hf
                    ps = psum_pool.tile([128, 4, TILE_PX], F32)
                    if USE_TILED:
                        for tg in range(2):        # tile pairs (0,2), (1,3)
                            for kwi in range(3):
                                for up in range(2):
                                    tp = tg + 2 * up
                                    h0 = (hf * 4 + tp) * ROWS_PER_TILE
                                    if up == 0:
                                        lhsT = wt[0:64, 3 + kwi, :]
                                        rhs = xx[0:64, h0 + 2:h0 + 10,
                                                 kwi:kwi + W]
                                    else:
                                        lhsT = wt[64:128, 3 + kwi, :]
                                        rhs = xx[64:128, h0 + 1:h0 + 9,
                                                 kwi:kwi + W]
                                    nc.tensor.matmul(
                                        ps[:, tp, :], lhsT=lhsT, rhs=rhs,
                                        start=(kwi == 0), stop=False,
                                        skip_group_check=True,
                                    )
                    else:
                        for kwi in range(3):
                            for tp in range(4):
                                h0 = (hf * 4 + tp) * ROWS_PER_TILE
                                nc.tensor.matmul(
                                    ps[:, tp, :],
                                    lhsT=wt[0:64, 3 + kwi, :],
                                    rhs=xx[0:64, h0 + 2:h0 + 10,
                                           kwi:kwi + W],
                                    start=(kwi == 0), stop=False,
                                    skip_group_check=True,
                                )
                    # pairs (kh=0,1), taps-outer for weight reuse
                    for kw in range(3):
                        for tp in range(4):
                            h0 = (hf * 4 + tp) * ROWS_PER_TILE
                            nc.tensor.matmul(
                                ps[:, tp, :], lhsT=wt[:, kw, :],
                                rhs=xx[:, h0:h0 + 8, kw:kw + W],
                                start=False, stop=(kw == 2),
                                skip_group_check=True,
                            )
                    gt4 = img * TILES_PER_IMG + hf * 4
                    # PSUM -> SBUF bf16 copy + per-channel sum (ScalarE)
                    nc.scalar.activation(
                        ybuf[:, gt4:gt4 + 4, :], ps[:],
                        mybir.ActivationFunctionType.Copy,
                        accum_out=sums[:, gh:gh + 1],
                    )
                    # square (bf16, 2x DVE rate) + per-channel sum of squares
                    sq = scratch_pool.tile([128, 4, TILE_PX], YDT)
                    nc.vector.tensor_mul(sq[:], ybuf[:, gt4:gt4 + 4, :],
                                         ybuf[:, gt4:gt4 + 4, :])
                    nc.vector.reduce_sum(sumsqs[:, gh:gh + 1], sq[:],
                                         axis=mybir.AxisListType.XY)

                if SPLIT_AG and img == 4:
                    # partial stats for images 0..4: gather them early so
                    # the collective (incl. ~15us ncfw wake) fully overlaps
                    # the remaining conv and the final gather hits a warm
                    # cc stream (measured 6.6us vs ~22us cold).
                    st_a = stats_pool.tile([128, 2], F32)
                    nc.vector.reduce_sum(st_a[:, 0:1], sums[:, 0:10],
                                         axis=mybir.AxisListType.X)
                    nc.vector.reduce_sum(st_a[:, 1:2], sumsqs[:, 0:10],
                                         axis=mybir.AxisListType.X)
                    nc.sync.dma_start(out=cc_in_a[:], in_=st_a[:])
                    nc.gpsimd.collective_compute(
                        "AllGather",
                        mybir.AluOpType.bypass,
                        ins=[cc_in_a[:]],
                        outs=[cc_out_a[:]],
                        replica_groups=[list(range(N_CORES))],
                    )
                    t8a = stats_pool.tile([128, 16], F32)
                    for r in range(N_CORES):
                        nc.sync.dma_start(out=t8a[:, 2 * r:2 * r + 2],
                                          in_=cc_out_a[r])

            # fold per-group partials, AllGather stats across the 8 cores
            st = stats_pool.tile([128, 2], F32)
            lo = 10 if SPLIT_AG else 0
            nc.vector.reduce_sum(st[:, 0:1], sums[:, lo:16],
                                 axis=mybir.AxisListType.X)
            nc.vector.reduce_sum(st[:, 1:2], sumsqs[:, lo:16],
                                 axis=mybir.AxisListType.X)
            nc.sync.dma_start(out=cc_in[:], in_=st[:])
            g = stats_pool.tile([128, 2], F32)
            if USE_AG:
                nc.gpsimd.collective_compute(
                    "AllGather",
                    mybir.AluOpType.bypass,
                    ins=[cc_in[:]],
                    outs=[cc_out[:]],
                    replica_groups=[list(range(N_CORES))],
                )
                # land each rank's [128,2] block side by side, tree-add
                t8 = stats_pool.tile([128, 16], F32)
                for r in range(N_CORES):
                    nc.sync.dma_start(out=t8[:, 2 * r:2 * r + 2],
                                      in_=cc_out[r])
                if SPLIT_AG:
                    t16 = stats_pool.tile([128, 16], F32)
                    nc.vector.tensor_add(t16[:], t8[:], t8a[:])
                    t8 = t16
                t4 = stats_pool.tile([128, 8], F32)
                nc.vector.tensor_add(t4[:], t8[:, 0:8], t8[:, 8:16])
                t2 = stats_pool.tile([128, 4], F32)
                nc.vector.tensor_add(t2[:], t4[:, 0:4], t4[:, 4:8])
                nc.vector.tensor_add(g[:], t2[:, 0:2], t2[:, 2:4])
            else:
                nc.gpsimd.collective_compute(
                    "AllReduce",
                    mybir.AluOpType.add,
                    ins=[cc_in[:]],
                    outs=[cc_ar[:]],
                    replica_groups=[list(range(N_CORES))],
                )
                nc.sync.dma_start(out=g[:], in_=cc_ar[:])

            # scale = gamma * rsqrt(var + eps); shift = beta - scale * mean
            m = stats_pool.tile([128, 2], F32)   # mean, E[y^2]
            var = stats_pool.tile([128, 1], F32)
            sd = stats_pool.tile([128, 1], F32)
            inv = stats_pool.tile([128, 1], F32)
            scl = stats_pool.tile([128, 1], F32)
            shv = stats_pool.tile([128, 1], F32)
            tmp = stats_pool.tile([128, 1], F32)
            nc.vector.tensor_scalar_mul(m[:], g[:], 1.0 / COUNT)
            nc.vector.tensor_mul(tmp[:], m[:, 0:1], m[:, 0:1])
            nc.vector.tensor_sub(var[:], m[:, 1:2], tmp[:])
            nc.scalar.activation(sd[:], var[:],
                                 mybir.ActivationFunctionType.Sqrt,
                                 bias=eps_t[:])
            nc.vector.reciprocal(inv[:], sd[:])
            nc.vector.tensor_mul(scl[:], gb[:, 0:1], inv[:])
            nc.vector.tensor_mul(tmp[:], scl[:], m[:, 0:1])
            nc.vector.tensor_sub(shv[:], gb[:, 1:2], tmp[:])

            # apply: out = relu(y * scale + shift), in half-image chunks
            CH_TILES = 4  # tiles per chunk
            for img in range(IMG_PER_CORE):
                for half in range(TILES_PER_IMG // CH_TILES):
                    t0 = img * TILES_PER_IMG + half * CH_TILES
                    ot = out_pool.tile([128, CH_TILES, TILE_PX], F32)
                    nc.scalar.activation(
                        ot[:], ybuf[:, t0:t0 + CH_TILES, :],
                        mybir.ActivationFunctionType.Relu,
                        bias=shv[:], scale=scl[:],
                    )
                    # one dma_start lands on a single ~25 GB/s DMA engine;
                    # split 2 ways (sync ring only -- DMA issue consumes
                    # engine time, and ScalarE is busy with activations)
                    px0 = half * CH_TILES * TILE_PX
                    for q in range(2):
                        p0 = px0 + q * 2 * TILE_PX
                        nc.sync.dma_start(
                            out=out_d[img, :, p0:p0 + 2 * TILE_PX],
                            in_=ot[:, 2 * q:2 * q + 2, :],
                        )

    nc.compile()
    return nc


def _prep_weights(weight: np.ndarray) -> np.ndarray:
    # [p, q, mb, mb] block matrix -> truncated OIHW kernel [128, 64, 3, 3]
    p, q, mb, _ = weight.shape
    Wm = weight.transpose(0, 2, 1, 3).reshape(p * mb, q * mb)
    Wm = Wm[:C_OUT, :C_IN * 9].reshape(C_OUT, C_IN, 3, 3)
    wt = np.zeros((128, 6, 128), np.float32)
    # pairs: partition c -> (kh=0), partition 64+c -> (kh=1)
    wt[:64, 0:3, :] = Wm[:, :, 0, :].transpose(1, 2, 0)
    wt[64:, 0:3, :] = Wm[:, :, 1, :].transpose(1, 2, 0)
    # singles (kh=2), duplicated in both partition halves
    wt[:64, 3:6, :] = Wm[:, :, 2, :].transpose(1, 2, 0)
    wt[64:, 3:6, :] = Wm[:, :, 2, :].transpose(1, 2, 0)
    import ml_dtypes
    return wt.astype(np.dtype(ml_dtypes.bfloat16))


def kernel(x, weight, gamma, beta):
    global _cached_nc, LAST_EXEC_NS
    x = np.asarray(x, np.float32)
    weight = np.asarray(weight, np.float32)
    gamma = np.asarray(gamma, np.float32)
    beta = np.asarray(beta, np.float32)

    if _cached_nc is None:
        _cached_nc = _build()
    nc = _cached_nc

    wt = _prep_weights(weight)
    gb = np.ascontiguousarray(np.stack([gamma, beta], axis=1))
    import ml_dtypes
    bf16 = np.dtype(ml_dtypes.bfloat16)
    xp = np.zeros((64, 128, HP * WP), bf16)
    pad = np.zeros((64, C_IN, HP, WP), np.float32)
    pad[:, :, 1:H + 1, 1:W + 1] = x
    pad = pad.reshape(64, C_IN, HP * WP).astype(bf16)
    xp[:, :C_IN, :] = pad
    xp[:, C_IN:, :HP * WP - WP] = pad[:, :, WP:]
    in_maps = []
    for i in range(N_CORES):
        shard = np.ascontiguousarray(
            xp[i * IMG_PER_CORE:(i + 1) * IMG_PER_CORE])
        in_maps.append({"x": shard, "wt": wt, "gb": gb})

    res = run_bass_kernel_spmd(nc, in_maps, list(range(N_CORES)),
                               trace=KERNEL_TRACE)
    LAST_EXEC_NS = res.exec_time_ns
    global LAST_TRACE_PATH, LAST_PROFILE_JSON
    if res.instructions_and_trace:
        LAST_TRACE_PATH = res.instructions_and_trace[1]
    LAST_PROFILE_JSON = res.profile_json

    out = np.concatenate(
        [res.results[i]["out"].reshape(IMG_PER_CORE, C_OUT, H, W)
         for i in range(N_CORES)], axis=0)
    return out


# revision 29
# speedup vs baseline: 1.2190x; 1.2190x over previous
"""Fused Conv3x3 + BatchNorm(train) + ReLU on 8 TRN2 NeuronCores.

Data-parallel over batch: each core processes 8 of the 64 images.
Conv is computed as matmuls over PSUM tiles of [128 out_ch, 512 pixels]:
the 9 filter taps are covered per tile by 3 K=128 matmuls (kh=0,1 paired
on the partition axis) plus 3 K=64 matmuls (kh=2) that are row-tiled so
two of them run concurrently in the PE array (rows 0-63 / rows 64-127)
-> ~5 effective 512-px streaming slots per tile instead of 6.

y is kept resident in SBUF as bf16. Per group the PSUM drain is split:
ScalarE copies PSUM->ybuf(bf16) with accum_out giving per-channel sums;
VectorE squares the bf16 y (2x DVE rate) and reduces to sum-of-squares.

Cross-core BN stats use an AllGather (floor ~5us vs ~27us measured for
AllReduce) followed by a local 3-step tree add. Scale/shift use a single
Rsqrt activation. Apply = relu(y*scale+shift) on ScalarE in chunks,
DMA-out overlapped (the tail is HBM-write-bound).
"""

import os

import numpy as np

import concourse.bacc as bacc
import concourse.tile as tile
from concourse import mybir
from concourse.bass_utils import run_bass_kernel_spmd

N_CORES = 8
IMG_PER_CORE = 8          # 64 images / 8 cores
C_IN = 64
C_OUT = 128
H = W = 64
HP, WP = H + 2, W + 2     # zero-padded image
PIX = H * W               # 4096
TILE_PX = 512             # one PSUM bank of fp32
ROWS_PER_TILE = TILE_PX // W       # 8
TILES_PER_IMG = PIX // TILE_PX     # 8
N_TILES = IMG_PER_CORE * TILES_PER_IMG  # 64
BN_EPS = 1e-5
COUNT = 64 * H * W        # batch-stat count over (N, H, W)

F32 = mybir.dt.float32
BF16 = mybir.dt.bfloat16

# Set by test harness to capture a profile; LAST_EXEC_NS holds the result.
KERNEL_TRACE = False
LAST_EXEC_NS = None
LAST_TRACE_PATH = None
LAST_PROFILE_JSON = None

_cached_nc = None

# bisect toggles (harness leaves these at defaults)
USE_TILED = os.environ.get("K_TILED", "1") == "1"   # row-tiled kh=2 taps
USE_AG = os.environ.get("K_AG", "1") == "1"         # AllGather vs AllReduce
USE_BF16Y = os.environ.get("K_BF16Y", "1") == "1"   # bf16 ybuf
SPLIT_AG = os.environ.get("K_SPLIT_AG", "0") == "1"  # early partial AG


def _build():
    nc = bacc.Bacc("TRN2", target_bir_lowering=False, debug=False,
                   num_devices=N_CORES)

    x_in = nc.dram_tensor("x", [IMG_PER_CORE, 128, HP * WP], BF16,
                          kind="ExternalInput")
    wt_in = nc.dram_tensor("wt", [128, 6, 128], BF16, kind="ExternalInput")
    gb_in = nc.dram_tensor("gb", [128, 2], F32, kind="ExternalInput")
    out_d = nc.dram_tensor("out", [IMG_PER_CORE, C_OUT, PIX], F32,
                           kind="ExternalOutput")
    cc_in = nc.dram_tensor("cc_in", [128, 2], F32)
    cc_out = nc.dram_tensor("cc_out", [N_CORES, 128, 2], F32,
                            addr_space="Shared")
    cc_ar = nc.dram_tensor("cc_ar", [128, 2], F32, addr_space="Shared")
    cc_in_a = nc.dram_tensor("cc_in_a", [128, 2], F32)
    cc_out_a = nc.dram_tensor("cc_out_a", [N_CORES, 128, 2], F32,
                              addr_space="Shared")

    with tile.TileContext(nc) as tc:
        with (
            tc.tile_pool(name="consts", bufs=1) as consts,
            tc.tile_pool(name="xx", bufs=2) as xx_pool,
            tc.tile_pool(name="ybuf", bufs=1) as ybuf_pool,
            tc.tile_pool(name="scratch", bufs=2) as scratch_pool,
            tc.tile_pool(name="stats", bufs=1) as stats_pool,
            tc.tile_pool(name="outp", bufs=3) as out_pool,
            tc.tile_pool(name="psum", bufs=2, space="PSUM") as psum_pool,
        ):
            wt = consts.tile([128, 6, 128], BF16)
            nc.sync.dma_start(out=wt[:], in_=wt_in[:])
            gb = consts.tile([128, 2], F32)
            nc.sync.dma_start(out=gb[:], in_=gb_in[:])
            eps_t = consts.tile([128, 1], F32)
            nc.vector.memset(eps_t[:], BN_EPS)

            # y stays resident in SBUF (bf16) between conv and BN apply.
            YDT = BF16 if USE_BF16Y else F32
            ybuf = ybuf_pool.tile([128, N_TILES, TILE_PX], YDT)
            sums = stats_pool.tile([128, N_TILES // 4], F32)
            sumsqs = stats_pool.tile([128, N_TILES // 4], F32)

            for img in range(IMG_PER_CORE):
                # xx: padded image, channels on partitions 0-63; partitions
                # 64-127 hold the same image shifted down one padded row so
                # (kh, kh+1) taps pair into one K=128 contraction.
                xx = xx_pool.tile([128, HP, WP], BF16)
                # split at the half-image boundary (row 35): the hf=0
                # matmuls only read rows 0-34, so they can start as soon
                # as the first chunk lands. Two HWDGE queues in parallel.
                cut = 35 * WP
                xv = xx[:, :, :].rearrange("p a b -> p (a b)")
                nc.sync.dma_start(out=xv[:, 0:cut], in_=x_in[img, :, 0:cut])
                nc.sync.dma_start(out=xv[:, cut:HP * WP],
                                  in_=x_in[img, :, cut:HP * WP])

                # Singles (kh=2) first so K never shrinks within a PSUM
                # bank. All kh=2 taps are readable from either image copy,
                # so output tiles 0-1 take theirs from the lower copy (PE
                # rows 0-63) and tiles 2-3 from the row-shifted upper copy
                # (rows 64-127): disjoint row groups AND disjoint PSUM
                # banks, so the PE streams both concurrently (race-free).
                # Issue order interleaves the two row groups.
                for hf in range(2):
                    gh = img * 2 + hf
                    ps = psum_pool.tile([128, 4, TILE_PX], F32)
                    if USE_TILED:
                        for tg in range(2):        # tile pairs (0,2), (1,3)
                            for kwi in range(3):
                                for up in range(2):
                                    tp = tg + 2 * up
                                    h0 = (hf * 4 + tp) * ROWS_PER_TILE
                                    if up == 0:
                                        lhsT = wt[0:64, 3 + kwi, :]
                                        rhs = xx[0:64, h0 + 2:h0 + 10,
                                                 kwi:kwi + W]
                                    else:
                                        lhsT = wt[64:128, 3 + kwi, :]
                                        rhs = xx[64:128, h0 + 1:h0 + 9,
                                                 kwi:kwi + W]
                                    nc.tensor.matmul(
                                        ps[:, tp, :], lhsT=lhsT, rhs=rhs,
                                        start=(kwi == 0), stop=False,
                                        skip_group_check=True,
                                    )
                    else:
                        for kwi in range(3):
                            for tp in range(4):
                                h0 = (hf * 4 + tp) * ROWS_PER_TILE
                                nc.tensor.matmul(
                                    ps[:, tp, :],
                                    lhsT=wt[0:64, 3 + kwi, :],
                                    rhs=xx[0:64, h0 + 2:h0 + 10,
                                           kwi:kwi + W],
                                    start=(kwi == 0), stop=False,
                                    skip_group_check=True,
                                )
                    # pairs (kh=0,1), taps-outer for weight reuse
                    for kw in range(3):
                        for tp in range(4):
                            h0 = (hf * 4 + tp) * ROWS_PER_TILE
                            nc.tensor.matmul(
                                ps[:, tp, :], lhsT=wt[:, kw, :],
                                rhs=xx[:, h0:h0 + 8, kw:kw + W],
                                start=False, stop=(kw == 2),
                                skip_group_check=True,
                            )
                    gt4 = img * TILES_PER_IMG + hf * 4
                    # PSUM -> SBUF bf16 copy + per-channel sum (ScalarE)
                    nc.scalar.activation(
                        ybuf[:, gt4:gt4 + 4, :], ps[:],
                        mybir.ActivationFunctionType.Copy,
                        accum_out=sums[:, gh:gh + 1],
                    )
                    # square (bf16, 2x DVE rate) + per-channel sum of squares
                    sq = scratch_pool.tile([128, 4, TILE_PX], YDT)
                    nc.vector.tensor_mul(sq[:], ybuf[:, gt4:gt4 + 4, :],
                                         ybuf[:, gt4:gt4 + 4, :])
                    nc.vector.reduce_sum(sumsqs[:, gh:gh + 1], sq[:],
                                         axis=mybir.AxisListType.XY)

                if SPLIT_AG and img == 4:
                    # partial stats for images 0..4: gather them early so
                    # the collective (incl. ~15us ncfw wake) fully overlaps
                    # the remaining conv and the final gather hits a warm
                    # cc stream (measured 6.6us vs ~22us cold).
                    st_a = stats_pool.tile([128, 2], F32)
                    nc.vector.reduce_sum(st_a[:, 0:1], sums[:, 0:10],
                                         axis=mybir.AxisListType.X)
                    nc.vector.reduce_sum(st_a[:, 1:2], sumsqs[:, 0:10],
                                         axis=mybir.AxisListType.X)
                    nc.sync.dma_start(out=cc_in_a[:], in_=st_a[:])
                    nc.gpsimd.collective_compute(
                        "AllGather",
                        mybir.AluOpType.bypass,
                        ins=[cc_in_a[:]],
                        outs=[cc_out_a[:]],
                        replica_groups=[list(range(N_CORES))],
                    )
                    t8a = stats_pool.tile([128, 16], F32)
                    for r in range(N_CORES):
                        nc.sync.dma_start(out=t8a[:, 2 * r:2 * r + 2],
                                          in_=cc_out_a[r])

            # fold per-group partials, AllGather stats across the 8 cores
            st = stats_pool.tile([128, 2], F32)
            lo = 10 if SPLIT_AG else 0
            nc.vector.reduce_sum(st[:, 0:1], sums[:, lo:16],
                                 axis=mybir.AxisListType.X)
            nc.vector.reduce_sum(st[:, 1:2], sumsqs[:, lo:16],
                                 axis=mybir.AxisListType.X)
            nc.sync.dma_start(out=cc_in[:], in_=st[:])
            g = stats_pool.tile([128, 2], F32)
            if USE_AG:
                nc.gpsimd.collective_compute(
                    "AllGather",
                    mybir.AluOpType.bypass,
                    ins=[cc_in[:]],
                    outs=[cc_out[:]],
                    replica_groups=[list(range(N_CORES))],
                )
                # land each rank's [128,2] block side by side, tree-add.
                # Split across both HWDGE rings: ScalarE is idle in this
                # window (conv copies done, apply not started), and the 8
                # serialized issues (~0.6us each) sit on the critical path.
                t8 = stats_pool.tile([128, 16], F32)
                for r in range(N_CORES):
                    eng = nc.sync if r % 2 == 0 else nc.scalar
                    eng.dma_start(out=t8[:, 2 * r:2 * r + 2],
                                  in_=cc_out[r])
                if SPLIT_AG:
                    t16 = stats_pool.tile([128, 16], F32)
                    nc.vector.tensor_add(t16[:], t8[:], t8a[:])
                    t8 = t16
                t4 = stats_pool.tile([128, 8], F32)
                nc.vector.tensor_add(t4[:], t8[:, 0:8], t8[:, 8:16])
                t2 = stats_pool.tile([128, 4], F32)
                nc.vector.tensor_add(t2[:], t4[:, 0:4], t4[:, 4:8])
                nc.vector.tensor_add(g[:], t2[:, 0:2], t2[:, 2:4])
            else:
                nc.gpsimd.collective_compute(
                    "AllReduce",
                    mybir.AluOpType.add,
                    ins=[cc_in[:]],
                    outs=[cc_ar[:]],
                    replica_groups=[list(range(N_CORES))],
                )
                nc.sync.dma_start(out=g[:], in_=cc_ar[:])

            # scale = gamma * rsqrt(var + eps); shift = beta - scale * mean
            m = stats_pool.tile([128, 2], F32)   # mean, E[y^2]
            var = stats_pool.tile([128, 1], F32)
            sd = stats_pool.tile([128, 1], F32)
            inv = stats_pool.tile([128, 1], F32)
            scl = stats_pool.tile([128, 1], F32)
            shv = stats_pool.tile([128, 1], F32)
            tmp = stats_pool.tile([128, 1], F32)
            nc.vector.tensor_scalar_mul(m[:], g[:], 1.0 / COUNT)
            nc.vector.tensor_mul(tmp[:], m[:, 0:1], m[:, 0:1])
            nc.vector.tensor_sub(var[:], m[:, 1:2], tmp[:])
            nc.scalar.activation(sd[:], var[:],
                                 mybir.ActivationFunctionType.Sqrt,
                                 bias=eps_t[:])
            nc.vector.reciprocal(inv[:], sd[:])
            nc.vector.tensor_mul(scl[:], gb[:, 0:1], inv[:])
            nc.vector.tensor_mul(tmp[:], scl[:], m[:, 0:1])
            nc.vector.tensor_sub(shv[:], gb[:, 1:2], tmp[:])

            # apply: out = relu(y * scale + shift), in half-image chunks.
            # First two chunks are half-size so the first output DMA (the
            # pacer of the HBM-write-bound tail) issues ~1us sooner.
            chunks = [(0, 2), (2, 2)] + [
                (t0, 4) for t0 in range(4, N_TILES, 4)]
            for t0, ct in chunks:
                img, px0 = t0 // TILES_PER_IMG, (t0 % TILES_PER_IMG) * TILE_PX
                ot = out_pool.tile([128, 4, TILE_PX], F32)
                nc.scalar.activation(
                    ot[:, 0:ct, :], ybuf[:, t0:t0 + ct, :],
                    mybir.ActivationFunctionType.Relu,
                    bias=shv[:], scale=scl[:],
                )
                # one dma_start lands on a single ~25 GB/s DMA engine;
                # split 2 ways (sync ring only -- DMA issue consumes
                # engine time, and ScalarE is busy with activations)
                for q in range(0, ct, 2):
                    p0 = px0 + q * TILE_PX
                    nc.sync.dma_start(
                        out=out_d[img, :, p0:p0 + 2 * TILE_PX],
                        in_=ot[:, q:q + 2, :],
                    )

    nc.compile()
    return nc


def _prep_weights(weight: np.ndarray) -> np.ndarray:
    # [p, q, mb, mb] block matrix -> truncated OIHW kernel [128, 64, 3, 3]
    p, q, mb, _ = weight.shape
    Wm = weight.transpose(0, 2, 1, 3).reshape(p * mb, q * mb)
    Wm = Wm[:C_OUT, :C_IN * 9].reshape(C_OUT, C_IN, 3, 3)
    wt = np.zeros((128, 6, 128), np.float32)
    # pairs: partition c -> (kh=0), partition 64+c -> (kh=1)
    wt[:64, 0:3, :] = Wm[:, :, 0, :].transpose(1, 2, 0)
    wt[64:, 0:3, :] = Wm[:, :, 1, :].transpose(1, 2, 0)
    # singles (kh=2), duplicated in both partition halves
    wt[:64, 3:6, :] = Wm[:, :, 2, :].transpose(1, 2, 0)
    wt[64:, 3:6, :] = Wm[:, :, 2, :].transpose(1, 2, 0)
    import ml_dtypes
    return wt.astype(np.dtype(ml_dtypes.bfloat16))


def kernel(x, weight, gamma, beta):
    global _cached_nc, LAST_EXEC_NS
    x = np.asarray(x, np.float32)
    weight = np.asarray(weight, np.float32)
    gamma = np.asarray(gamma, np.float32)
    beta = np.asarray(beta, np.float32)

    if _cached_nc is None:
        _cached_nc = _build()
    nc = _cached_nc

    wt = _prep_weights(weight)
    gb = np.ascontiguousarray(np.stack([gamma, beta], axis=1))
    import ml_dtypes
    bf16 = np.dtype(ml_dtypes.bfloat16)
    xp = np.zeros((64, 128, HP * WP), bf16)
    pad = np.zeros((64, C_IN, HP, WP), np.float32)
    pad[:, :, 1:H + 1, 1:W + 1] = x
    pad = pad.reshape(64, C_IN, HP * WP).astype(bf16)
    xp[:, :C_IN, :] = pad
    xp[:, C_IN:, :HP * WP - WP] = pad[:, :, WP:]
    in_maps = []
    for i in range(N_CORES):
        shard = np.ascontiguousarray(
            xp[i * IMG_PER_CORE:(i + 1) * IMG_PER_CORE])
        in_maps.append({"x": shard, "wt": wt, "gb": gb})

    res = run_bass_kernel_spmd(nc, in_maps, list(range(N_CORES)),
                               trace=KERNEL_TRACE)
    LAST_EXEC_NS = res.exec_time_ns
    global LAST_TRACE_PATH, LAST_PROFILE_JSON
    if res.instructions_and_trace:
        LAST_TRACE_PATH = res.instructions_and_trace[1]
    LAST_PROFILE_JSON = res.profile_json

    out = np.concatenate(
        [res.results[i]["out"].reshape(IMG_PER_CORE, C_OUT, H, W)
         for i in range(N_CORES)], axis=0)
    return out


# revision 30
# speedup vs baseline: 1.2616x; 1.0350x over previous
"""Fused Conv3x3 + BatchNorm(train) + ReLU on 8 TRN2 NeuronCores.

Data-parallel over batch: each core processes 8 of the 64 images.
Conv is computed as matmuls over PSUM tiles of [128 out_ch, 512 pixels]:
the 9 filter taps are covered per tile by 3 K=128 matmuls (kh=0,1 paired
on the partition axis) plus 3 K=64 matmuls (kh=2) that are row-tiled so
two of them run concurrently in the PE array (rows 0-63 / rows 64-127)
-> ~5 effective 512-px streaming slots per tile instead of 6.

y is kept resident in SBUF as bf16. Per group the PSUM drain is split:
ScalarE copies PSUM->ybuf(bf16) with accum_out giving per-channel sums;
VectorE squares the bf16 y (2x DVE rate) and reduces to sum-of-squares.

Cross-core BN stats use an AllGather (floor ~5us vs ~27us measured for
AllReduce) followed by a local 3-step tree add. Scale/shift use a single
Rsqrt activation. Apply = relu(y*scale+shift) on ScalarE in chunks,
DMA-out overlapped (the tail is HBM-write-bound).
"""

import os

import numpy as np

import concourse.bacc as bacc
import concourse.tile as tile
from concourse import mybir
from concourse.bass_utils import run_bass_kernel_spmd

N_CORES = 8
IMG_PER_CORE = 8          # 64 images / 8 cores
C_IN = 64
C_OUT = 128
H = W = 64
HP, WP = H + 2, W + 2     # zero-padded image
PIX = H * W               # 4096
TILE_PX = 512             # one PSUM bank of fp32
ROWS_PER_TILE = TILE_PX // W       # 8
TILES_PER_IMG = PIX // TILE_PX     # 8
N_TILES = IMG_PER_CORE * TILES_PER_IMG  # 64
BN_EPS = 1e-5
COUNT = 64 * H * W        # batch-stat count over (N, H, W)

F32 = mybir.dt.float32
BF16 = mybir.dt.bfloat16

# Set by test harness to capture a profile; LAST_EXEC_NS holds the result.
KERNEL_TRACE = False
LAST_EXEC_NS = None
LAST_TRACE_PATH = None
LAST_PROFILE_JSON = None

_cached_nc = None

# bisect toggles (harness leaves these at defaults)
USE_TILED = os.environ.get("K_TILED", "1") == "1"   # row-tiled kh=2 taps
USE_AG = os.environ.get("K_AG", "1") == "1"         # AllGather vs AllReduce
USE_BF16Y = os.environ.get("K_BF16Y", "1") == "1"   # bf16 ybuf
SPLIT_AG = os.environ.get("K_SPLIT_AG", "0") == "1"  # early partial AG


def _build():
    nc = bacc.Bacc("TRN2", target_bir_lowering=False, debug=False,
                   num_devices=N_CORES)

    x_in = nc.dram_tensor("x", [IMG_PER_CORE, 128, HP * WP], BF16,
                          kind="ExternalInput")
    wt_in = nc.dram_tensor("wt", [128, 6, 128], BF16, kind="ExternalInput")
    gb_in = nc.dram_tensor("gb", [128, 2], F32, kind="ExternalInput")
    out_d = nc.dram_tensor("out", [IMG_PER_CORE, C_OUT, PIX], F32,
                           kind="ExternalOutput")
    cc_in = nc.dram_tensor("cc_in", [128, 2], F32)
    cc_out = nc.dram_tensor("cc_out", [N_CORES, 128, 2], F32,
                            addr_space="Shared")
    cc_ar = nc.dram_tensor("cc_ar", [128, 2], F32, addr_space="Shared")
    cc_in_a = nc.dram_tensor("cc_in_a", [128, 2], F32)
    cc_out_a = nc.dram_tensor("cc_out_a", [N_CORES, 128, 2], F32,
                              addr_space="Shared")

    with tile.TileContext(nc) as tc:
        with (
            tc.tile_pool(name="consts", bufs=1) as consts,
            tc.tile_pool(name="xx", bufs=2) as xx_pool,
            tc.tile_pool(name="ybuf", bufs=1) as ybuf_pool,
            tc.tile_pool(name="scratch", bufs=2) as scratch_pool,
            tc.tile_pool(name="stats", bufs=1) as stats_pool,
            tc.tile_pool(name="outp", bufs=3) as out_pool,
            tc.tile_pool(name="psum", bufs=2, space="PSUM") as psum_pool,
        ):
            wt = consts.tile([128, 6, 128], BF16)
            nc.sync.dma_start(out=wt[:], in_=wt_in[:])
            gb = consts.tile([128, 2], F32)
            nc.sync.dma_start(out=gb[:], in_=gb_in[:])
            eps_t = consts.tile([128, 1], F32)
            nc.vector.memset(eps_t[:], BN_EPS)

            # y stays resident in SBUF (bf16) between conv and BN apply.
            YDT = BF16 if USE_BF16Y else F32
            ybuf = ybuf_pool.tile([128, N_TILES, TILE_PX], YDT)
            sums = stats_pool.tile([128, N_TILES // 4], F32)
            sumsqs = stats_pool.tile([128, N_TILES // 4], F32)

            for img in range(IMG_PER_CORE):
                # xx: padded image, channels on partitions 0-63; partitions
                # 64-127 hold the same image shifted down one padded row so
                # (kh, kh+1) taps pair into one K=128 contraction.
                xx = xx_pool.tile([128, HP, WP], BF16)
                # 4-way split: each dma_start binds to one ~25.6 GB/s DMA
                # engine, and per-image load (2-way) was ~11us vs ~8.6us of
                # compute -- the conv was input-paced. Four chunks bring the
                # load under compute. Boundaries respect reader ranges
                # (hf=0 reads rows 0-34, hf=1 rows 32-65).
                cuts = [0, 18 * WP, 35 * WP, 51 * WP, HP * WP]
                xv = xx[:, :, :].rearrange("p a b -> p (a b)")
                for c0, c1 in zip(cuts, cuts[1:]):
                    nc.sync.dma_start(out=xv[:, c0:c1],
                                      in_=x_in[img, :, c0:c1])

                # Singles (kh=2) first so K never shrinks within a PSUM
                # bank. All kh=2 taps are readable from either image copy,
                # so output tiles 0-1 take theirs from the lower copy (PE
                # rows 0-63) and tiles 2-3 from the row-shifted upper copy
                # (rows 64-127): disjoint row groups AND disjoint PSUM
                # banks, so the PE streams both concurrently (race-free).
                # Issue order interleaves the two row groups.
                for hf in range(2):
                    gh = img * 2 + hf
                    ps = psum_pool.tile([128, 4, TILE_PX], F32)
                    if USE_TILED:
                        for tg in range(2):        # tile pairs (0,2), (1,3)
                            for kwi in range(3):
                                for up in range(2):
                                    tp = tg + 2 * up
                                    h0 = (hf * 4 + tp) * ROWS_PER_TILE
                                    if up == 0:
                                        lhsT = wt[0:64, 3 + kwi, :]
                                        rhs = xx[0:64, h0 + 2:h0 + 10,
                                                 kwi:kwi + W]
                                    else:
                                        lhsT = wt[64:128, 3 + kwi, :]
                                        rhs = xx[64:128, h0 + 1:h0 + 9,
                                                 kwi:kwi + W]
                                    nc.tensor.matmul(
                                        ps[:, tp, :], lhsT=lhsT, rhs=rhs,
                                        start=(kwi == 0), stop=False,
                                        skip_group_check=True,
                                    )
                    else:
                        for kwi in range(3):
                            for tp in range(4):
                                h0 = (hf * 4 + tp) * ROWS_PER_TILE
                                nc.tensor.matmul(
                                    ps[:, tp, :],
                                    lhsT=wt[0:64, 3 + kwi, :],
                                    rhs=xx[0:64, h0 + 2:h0 + 10,
                                           kwi:kwi + W],
                                    start=(kwi == 0), stop=False,
                                    skip_group_check=True,
                                )
                    # pairs (kh=0,1), taps-outer for weight reuse
                    for kw in range(3):
                        for tp in range(4):
                            h0 = (hf * 4 + tp) * ROWS_PER_TILE
                            nc.tensor.matmul(
                                ps[:, tp, :], lhsT=wt[:, kw, :],
                                rhs=xx[:, h0:h0 + 8, kw:kw + W],
                                start=False, stop=(kw == 2),
                                skip_group_check=True,
                            )
                    gt4 = img * TILES_PER_IMG + hf * 4
                    # PSUM -> SBUF bf16 copy + per-channel sum (ScalarE)
                    nc.scalar.activation(
                        ybuf[:, gt4:gt4 + 4, :], ps[:],
                        mybir.ActivationFunctionType.Copy,
                        accum_out=sums[:, gh:gh + 1],
                    )
                    # square (bf16, 2x DVE rate) + per-channel sum of squares
                    sq = scratch_pool.tile([128, 4, TILE_PX], YDT)
                    nc.vector.tensor_mul(sq[:], ybuf[:, gt4:gt4 + 4, :],
                                         ybuf[:, gt4:gt4 + 4, :])
                    nc.vector.reduce_sum(sumsqs[:, gh:gh + 1], sq[:],
                                         axis=mybir.AxisListType.XY)

                if SPLIT_AG and img == 4:
                    # partial stats for images 0..4: gather them early so
                    # the collective (incl. ~15us ncfw wake) fully overlaps
                    # the remaining conv and the final gather hits a warm
                    # cc stream (measured 6.6us vs ~22us cold).
                    st_a = stats_pool.tile([128, 2], F32)
                    nc.vector.reduce_sum(st_a[:, 0:1], sums[:, 0:10],
                                         axis=mybir.AxisListType.X)
                    nc.vector.reduce_sum(st_a[:, 1:2], sumsqs[:, 0:10],
                                         axis=mybir.AxisListType.X)
                    nc.sync.dma_start(out=cc_in_a[:], in_=st_a[:])
                    nc.gpsimd.collective_compute(
                        "AllGather",
                        mybir.AluOpType.bypass,
                        ins=[cc_in_a[:]],
                        outs=[cc_out_a[:]],
                        replica_groups=[list(range(N_CORES))],
                    )
                    t8a = stats_pool.tile([128, 16], F32)
                    for r in range(N_CORES):
                        nc.sync.dma_start(out=t8a[:, 2 * r:2 * r + 2],
                                          in_=cc_out_a[r])

            # fold per-group partials, AllGather stats across the 8 cores
            st = stats_pool.tile([128, 2], F32)
            lo = 10 if SPLIT_AG else 0
            nc.vector.reduce_sum(st[:, 0:1], sums[:, lo:16],
                                 axis=mybir.AxisListType.X)
            nc.vector.reduce_sum(st[:, 1:2], sumsqs[:, lo:16],
                                 axis=mybir.AxisListType.X)
            nc.sync.dma_start(out=cc_in[:], in_=st[:])
            g = stats_pool.tile([128, 2], F32)
            if USE_AG:
                nc.gpsimd.collective_compute(
                    "AllGather",
                    mybir.AluOpType.bypass,
                    ins=[cc_in[:]],
                    outs=[cc_out[:]],
                    replica_groups=[list(range(N_CORES))],
                )
                # land each rank's [128,2] block side by side, tree-add.
                # Split across both HWDGE rings: ScalarE is idle in this
                # window (conv copies done, apply not started), and the 8
                # serialized issues (~0.6us each) sit on the critical path.
                t8 = stats_pool.tile([128, 16], F32)
                for r in range(N_CORES):
                    eng = nc.sync if r % 2 == 0 else nc.scalar
                    eng.dma_start(out=t8[:, 2 * r:2 * r + 2],
                                  in_=cc_out[r])
                if SPLIT_AG:
                    t16 = stats_pool.tile([128, 16], F32)
                    nc.vector.tensor_add(t16[:], t8[:], t8a[:])
                    t8 = t16
                t4 = stats_pool.tile([128, 8], F32)
                nc.vector.tensor_add(t4[:], t8[:, 0:8], t8[:, 8:16])
                t2 = stats_pool.tile([128, 4], F32)
                nc.vector.tensor_add(t2[:], t4[:, 0:4], t4[:, 4:8])
                nc.vector.tensor_add(g[:], t2[:, 0:2], t2[:, 2:4])
            else:
                nc.gpsimd.collective_compute(
                    "AllReduce",
                    mybir.AluOpType.add,
                    ins=[cc_in[:]],
                    outs=[cc_ar[:]],
                    replica_groups=[list(range(N_CORES))],
                )
                nc.sync.dma_start(out=g[:], in_=cc_ar[:])

            # scale = gamma * rsqrt(var + eps); shift = beta - scale * mean
            m = stats_pool.tile([128, 2], F32)   # mean, E[y^2]
            var = stats_pool.tile([128, 1], F32)
            sd = stats_pool.tile([128, 1], F32)
            inv = stats_pool.tile([128, 1], F32)
            scl = stats_pool.tile([128, 1], F32)
            shv = stats_pool.tile([128, 1], F32)
            tmp = stats_pool.tile([128, 1], F32)
            nc.vector.tensor_scalar_mul(m[:], g[:], 1.0 / COUNT)
            nc.vector.tensor_mul(tmp[:], m[:, 0:1], m[:, 0:1])
            nc.vector.tensor_sub(var[:], m[:, 1:2], tmp[:])
            nc.scalar.activation(sd[:], var[:],
                                 mybir.ActivationFunctionType.Sqrt,
                                 bias=eps_t[:])
            nc.vector.reciprocal(inv[:], sd[:])
            nc.vector.tensor_mul(scl[:], gb[:, 0:1], inv[:])
            nc.vector.tensor_mul(tmp[:], scl[:], m[:, 0:1])
            nc.vector.tensor_sub(shv[:], gb[:, 1:2], tmp[:])

            # apply: out = relu(y * scale + shift), in half-image chunks.
            # First two chunks are half-size so the first output DMA (the
            # pacer of the HBM-write-bound tail) issues ~1us sooner.
            chunks = [(0, 2), (2, 2)] + [
                (t0, 4) for t0 in range(4, N_TILES, 4)]
            for t0, ct in chunks:
                img, px0 = t0 // TILES_PER_IMG, (t0 % TILES_PER_IMG) * TILE_PX
                ot = out_pool.tile([128, 4, TILE_PX], F32)
                nc.scalar.activation(
                    ot[:, 0:ct, :], ybuf[:, t0:t0 + ct, :],
                    mybir.ActivationFunctionType.Relu,
                    bias=shv[:], scale=scl[:],
                )
                # one dma_start lands on a single ~25 GB/s DMA engine;
                # split 2 ways (sync ring only -- DMA issue consumes
                # engine time, and ScalarE is busy with activations)
                for q in range(0, ct, 2):
                    p0 = px0 + q * TILE_PX
                    nc.sync.dma_start(
                        out=out_d[img, :, p0:p0 + 2 * TILE_PX],
                        in_=ot[:, q:q + 2, :],
                    )

    nc.compile()
    return nc


def _prep_weights(weight: np.ndarray) -> np.ndarray:
    # [p, q, mb, mb] block matrix -> truncated OIHW kernel [128, 64, 3, 3]
    p, q, mb, _ = weight.shape
    Wm = weight.transpose(0, 2, 1, 3).reshape(p * mb, q * mb)
    Wm = Wm[:C_OUT, :C_IN * 9].reshape(C_OUT, C_IN, 3, 3)
    wt = np.zeros((128, 6, 128), np.float32)
    # pairs: partition c -> (kh=0), partition 64+c -> (kh=1)
    wt[:64, 0:3, :] = Wm[:, :, 0, :].transpose(1, 2, 0)
    wt[64:, 0:3, :] = Wm[:, :, 1, :].transpose(1, 2, 0)
    # singles (kh=2), duplicated in both partition halves
    wt[:64, 3:6, :] = Wm[:, :, 2, :].transpose(1, 2, 0)
    wt[64:, 3:6, :] = Wm[:, :, 2, :].transpose(1, 2, 0)
    import ml_dtypes
    return wt.astype(np.dtype(ml_dtypes.bfloat16))


def kernel(x, weight, gamma, beta):
    global _cached_nc, LAST_EXEC_NS
    x = np.asarray(x, np.float32)
    weight = np.asarray(weight, np.float32)
    gamma = np.asarray(gamma, np.float32)
    beta = np.asarray(beta, np.float32)

    if _cached_nc is None:
        _cached_nc = _build()
    nc = _cached_nc

    wt = _prep_weights(weight)
    gb = np.ascontiguousarray(np.stack([gamma, beta], axis=1))
    import ml_dtypes
    bf16 = np.dtype(ml_dtypes.bfloat16)
    xp = np.zeros((64, 128, HP * WP), bf16)
    pad = np.zeros((64, C_IN, HP, WP), np.float32)
    pad[:, :, 1:H + 1, 1:W + 1] = x
    pad = pad.reshape(64, C_IN, HP * WP).astype(bf16)
    xp[:, :C_IN, :] = pad
    xp[:, C_IN:, :HP * WP - WP] = pad[:, :, WP:]
    in_maps = []
    for i in range(N_CORES):
        shard = np.ascontiguousarray(
            xp[i * IMG_PER_CORE:(i + 1) * IMG_PER_CORE])
        in_maps.append({"x": shard, "wt": wt, "gb": gb})

    res = run_bass_kernel_spmd(nc, in_maps, list(range(N_CORES)),
                               trace=KERNEL_TRACE)
    LAST_EXEC_NS = res.exec_time_ns
    global LAST_TRACE_PATH, LAST_PROFILE_JSON
    if res.instructions_and_trace:
        LAST_TRACE_PATH = res.instructions_and_trace[1]
    LAST_PROFILE_JSON = res.profile_json

    out = np.concatenate(
        [res.results[i]["out"].reshape(IMG_PER_CORE, C_OUT, H, W)
         for i in range(N_CORES)], axis=0)
    return out
